# revision 3
# baseline (speedup 1.0000x reference)
"""V5: two-level structural fold via the real-DFT cos/sin split, bf16.

Each stage transform y = A @ x (A[:, perm] = B, B[k,j] = c_k cos(2pi kj/N)
+ s_k sin(2pi kj/N)) is computed as y[k] = c_k Vc[k] + s_k Vs[k] with
Vc/Vs the cos/sin transforms of w = x[perm].  DFT symmetries (expk-free!)
fold the contraction twice:  j <-> N-j, then j <-> N/2-j with k-parity,
and Vc[N-k] = Vc[k], Vs[N-k] = -Vs[k] halves the output count.  Net MACs:
N^2/4 per 1D transform (half of the single-fold V3).

Layout: folded input has 4 blocks [PP 1025->1152 | PM 1024 | MM 1023->1024
| MP 1024] = 4224 rows (33 chunks of 128).  Weight grids are plain
cos/sin evaluations; expk enters only via on-device combines:
  stage1:  Mq_A = cA*Vc + sA*Vs,  Mq_B = cB*Vc + sB*Vs   (per-column)
  stage2:  out_A = c0A*V2c + s0A*V2s, out_B likewise      (per-partition)
Output rows/columns land in kernel-local order; the host permutes them
(and computes the single k=0 output column directly).
"""
import numpy as np
import ml_dtypes

BF16 = ml_dtypes.bfloat16

N = 4096
M = N // 2            # 2048
Q = N // 4            # 1024
P = 128
NF = 4224             # folded length: 1152 + 1024 + 1024 + 1024
NCH = NF // P         # 33 chunks
KC = 512              # output columns per core (= 256 k-pairs)
NCORES = 8

# global chunk bases of the 4 blocks
PP0, PM0, MM0, MP0 = 0, 9, 17, 25
CHUNKS = {"PP": (PP0, 9), "PM": (PM0, 8), "MM": (MM0, 8), "MP": (MP0, 8)}

_NC_CACHE = {}


def _makhoul_perm(n):
    j = np.arange(n)
    return np.where(j < n // 2, 2 * j, 2 * (n - 1 - j) + 1)


def _build_A(expk, n):
    c = expk[:, 0].astype(np.float64)
    s = expk[:, 1].astype(np.float64)
    k = np.arange(n, dtype=np.int64)
    j = np.arange(n, dtype=np.int64)
    ang = (2.0 * np.pi / n) * ((k[:, None] * j[None, :]) % n).astype(np.float64)
    B = c[:, None] * np.cos(ang) + s[:, None] * np.sin(ang)
    A = np.empty((n, n), dtype=np.float64)
    A[:, _makhoul_perm(n)] = B
    return A.astype(np.float32)


def _fold2(v):
    """[4096, ...] (already permuted) -> [4224, ...] 4-block folded."""
    w = v
    p = w[1:M] + w[:M:-1]            # j=1..2047: w[j] + w[N-j]
    m_ = w[1:M] - w[:M:-1]
    pp = p[0:Q - 1] + p[2 * Q - 2:Q - 1:-1]   # j=1..1023 (+ partner 2047-j...)
    pm = p[0:Q - 1] - p[2 * Q - 2:Q - 1:-1]
    mm = m_[0:Q - 1] - m_[2 * Q - 2:Q - 1:-1]
    mp = m_[0:Q - 1] + m_[2 * Q - 2:Q - 1:-1]
    tail = v.shape[1:]
    z = np.zeros((1,) + tail, v.dtype)
    out = np.concatenate([
        (w[0] + w[M])[None], pp, p[Q - 1][None],          # PP: 1025
        np.zeros((127,) + tail, v.dtype),                 #   pad -> 1152
        (w[0] - w[M])[None], pm,                          # PM: 1024
        mm, z,                                            # MM: 1023 + pad
        mp, m_[Q - 1][None],                              # MP: 1024
    ], axis=0)
    assert out.shape[0] == NF
    return out


def _cos_grid(kvals, nrows, joff):
    j = (np.arange(nrows, dtype=np.float64) + joff)
    ang = 2.0 * np.pi / N * np.outer(j, kvals.astype(np.float64))
    return np.cos(ang)


def _sin_grid(kvals, nrows, joff):
    j = (np.arange(nrows, dtype=np.float64) + joff)
    ang = 2.0 * np.pi / N * np.outer(j, kvals.astype(np.float64))
    return np.sin(ang)


def _stage_weights(kE, kO):
    """[1152, 4*len] block-column weight matrix (zeros outside each block).

    Column groups: [VcE | VcO | VsE | VsO]; row r of group g is local row r
    of that group's fold block.  jidx: PP/PM -> r, MM/MP -> r+1.
    """
    L = len(kE)
    Wm = np.zeros((1152, 4 * L), dtype=np.float64)
    Wm[0:1025, 0:L] = _cos_grid(kE, 1025, 0)                  # VcE over PP
    Wm[0:1024, L:2 * L] = _cos_grid(kO, 1024, 0)              # VcO over PM
    Wm[0:1023, 2 * L:3 * L] = _sin_grid(kE, 1023, 1)          # VsE over MM
    Wm[0:1024, 3 * L:4 * L] = _sin_grid(kO, 1024, 1)          # VsO over MP
    return Wm.astype(np.float32)


def _klist(core):
    """k order for this core's columns: [E ascending | O ascending]."""
    base = 256 * core
    kE = np.arange(base + 2, base + 257, 2)
    kO = np.arange(base + 1, base + 256, 2)
    return kE, kO, np.concatenate([kE, kO])


_L_E = np.arange(0, 2304, 2)      # 1152 even l per stage-2 tile grid
_L_O = np.arange(1, 2304, 2)      # 1152 odd l


def _prep(x, expk0, expk1):
    x = np.asarray(x, dtype=np.float32)
    e0 = np.asarray(expk0, np.float32)
    e1 = np.asarray(expk1, np.float32)
    xt = np.ascontiguousarray(x.T)                       # [n, r]
    w = xt[_makhoul_perm(N), :]                          # stage-1 perm on n
    xf = _fold2(w)                                       # fold n -> [4224, r]
    wr = xf[:, _makhoul_perm(N)]                         # stage-2 perm on r
    xqq = _fold2(np.ascontiguousarray(wr.T)).T           # fold r -> [4224,4224]
    xqq = np.ascontiguousarray(xqq.astype(BF16))

    # stage-2 weights (shared by all cores): cols [V2cE|V2cO|V2sE|V2sO],
    # interleaved DMA layout [1152, par, t, {cos,sin}, 128]
    W0 = _stage_weights(_L_E, _L_O)                      # [1152, 4608] f32
    w0 = W0.reshape(1152, 4, 9, 128)                     # group, t, lane
    w0i = np.empty((1152, 2, 9, 2, 128), dtype=np.float32)
    w0i[:, 0, :, 0] = w0[:, 0]                           # par E: cos = V2cE
    w0i[:, 0, :, 1] = w0[:, 2]                           #        sin = V2sE
    w0i[:, 1, :, 0] = w0[:, 1]                           # par O: cos = V2cO
    w0i[:, 1, :, 1] = w0[:, 3]
    w0i = np.ascontiguousarray(
        w0i.reshape(1152, 2 * 9 * 2 * 128).astype(BF16))

    # stage-2 combine scalars [128, 2(par), 9(t), 2(set), 2(c/s)]
    csc0 = np.zeros((P, 2, 9, 2, 2), dtype=np.float32)
    pvec = np.arange(P)
    for par in range(2):
        for t in range(9):
            lr = 2 * (128 * t + pvec) + par
            vA = lr <= 2048
            lrA = np.where(vA, lr, 0)
            csc0[:, par, t, 0, 0] = np.where(vA, e0[lrA, 0], 0.0)
            csc0[:, par, t, 0, 1] = np.where(vA, e0[lrA, 1], 0.0)
            vB = (lr >= 1) & (lr <= 2047)
            lrB = np.where(vB, (N - lr) % N, 0)
            csc0[:, par, t, 1, 0] = np.where(vB, e0[lrB, 0], 0.0)
            csc0[:, par, t, 1, 1] = np.where(vB, -e0[lrB, 1], 0.0)
    csc0 = np.ascontiguousarray(csc0.reshape(P, 72))

    in_maps = []
    for c in range(NCORES):
        kE, kO, kl = _klist(c)
        w1 = np.ascontiguousarray(_stage_weights(kE, kO).astype(BF16))
        # stage-1 combine vectors [128(bcast), 4, 256]: cA, sA, cB, sB
        csc1 = np.empty((4, 256), dtype=np.float32)
        csc1[0] = e1[kl, 0]
        csc1[1] = e1[kl, 1]
        csc1[2] = e1[N - kl, 0]
        csc1[3] = -e1[N - kl, 1]
        csc1 = np.ascontiguousarray(
            np.broadcast_to(csc1.reshape(1, 4, 256), (P, 4, 256)).reshape(
                P, 1024).copy())
        in_maps.append({"xqq": xqq, "w1": w1, "w0": w0i,
                        "csc1": csc1, "csc0": csc0})
    return in_maps


def _assemble(x, expk0, expk1, core_outs):
    """core_outs[c]: [36, 128, 512] kernel-local -> full [4096, 4096]."""
    e0 = np.asarray(expk0, np.float32)
    out = np.empty((N, N), dtype=np.float32)
    pvec = np.arange(P)
    # row map: kernel tile (par, t) row p, set A -> l=lr, set B -> l=N-lr
    rowsA, rowsB, validA, validB = [], [], [], []
    for par in range(2):
        for t in range(9):
            lr = 2 * (128 * t + pvec) + par
            rowsA.append(lr)
            validA.append(lr <= 2048)
            rowsB.append((N - lr) % N)
            validB.append((lr >= 1) & (lr <= 2047))
    rowsA = np.concatenate(rowsA)
    rowsB = np.concatenate(rowsB)
    validA = np.concatenate(validA)
    validB = np.concatenate(validB)
    for c in range(NCORES):
        _, _, kl = _klist(c)
        kcols = np.concatenate([kl, (N - kl) % N])
        o = core_outs[c].reshape(18, 2, P, KC)   # [(par,t), set, p, col]
        oA = o[:, 0].reshape(18 * P, KC)
        oB = o[:, 1].reshape(18 * P, KC)
        out[np.ix_(rowsB[validB], kcols)] = oB[validB]
        out[np.ix_(rowsA[validA], kcols)] = oA[validA]
    # k=0 column directly (its Vc[0] has no slot in the pair sharding)
    A1 = _build_A(np.asarray(expk1, np.float32), N)
    A0 = _build_A(e0, N)
    m0 = np.asarray(x, np.float32) @ A1[0]
    out[:, 0] = A0 @ m0
    # col 2048 is produced by both core7/setA (exact) and core7/setB (dup);
    # setA was written last above per-core, but cores write disjoint column
    # sets except 2048 (core 7 A and B): A written after B within core 7. OK
    return out


def _host_sim(x, expk0, expk1):
    """Numpy sim of the kernel dataflow incl. bf16 rounding."""
    in_maps = _prep(x, expk0, expk1)
    outs = []
    for c in range(NCORES):
        mp_ = in_maps[c]
        xqq = mp_["xqq"].astype(np.float32)              # [4224, 4224]
        w1 = mp_["w1"].astype(np.float32)                # [1152, 512]
        csc1 = mp_["csc1"].reshape(P, 4, 256)[0]         # [4, 256]
        w0 = mp_["w0"].astype(np.float32).reshape(1152, 2, 9, 2, 128)
        csc0 = mp_["csc0"].reshape(P, 2, 9, 2, 2)
        # stage 1: psi[b, 512] with per-group block contractions
        W1g = np.zeros((NF, KC), dtype=np.float32)
        W1g[0:1152, 0:128] = w1[:, 0:128]
        W1g[1152:2176, 128:256] = w1[0:1024, 128:256]
        W1g[2176:3200, 256:384] = w1[0:1024, 256:384]
        W1g[3200:4224, 384:512] = w1[0:1024, 384:512]
        psi = xqq.T @ W1g                                # [4224 b, 512]
        vc, vs = psi[:, 0:256], psi[:, 256:512]
        mqA = (vc * csc1[0] + vs * csc1[1]).astype(BF16).astype(np.float32)
        mqB = (vc * csc1[2] + vs * csc1[3]).astype(BF16).astype(np.float32)
        mq = np.concatenate([mqA, mqB], axis=1)          # [4224, 512]
        # stage 2
        out = np.empty((36, P, KC), dtype=np.float32)
        base = {0: (PP0, 9, MM0, 8), 1: (PM0, 8, MP0, 8)}
        for par in range(2):
            cb, cn, sb, sn = base[par]
            for t in range(9):
                wc = w0[:, par, t, 0]                    # [1152, 128]
                ws = w0[:, par, t, 1]
                pc = wc[0:cn * 128].T @ mq[cb * 128:(cb + cn) * 128]
                ps = ws[0:sn * 128].T @ mq[sb * 128:(sb + sn) * 128]
                for st in range(2):
                    cs = csc0[:, par, t, st]             # [128, 2]
                    out[(par * 9 + t) * 2 + st] = (
                        pc * cs[:, 0:1] + ps * cs[:, 1:2])
        outs.append(out)
    return _assemble(x, expk0, expk1, outs)


def _build_nc(reps=1):
    import concourse.bacc as bacc
    import concourse.mybir as mybir
    import concourse.tile as tile

    BF = mybir.dt.bfloat16
    FP32 = mybir.dt.float32
    AF = mybir.ActivationFunctionType
    OP = mybir.AluOpType
    nc = bacc.Bacc("TRN2", target_bir_lowering=False, debug=False,
                   num_devices=NCORES)

    xqq_d = nc.dram_tensor("xqq", [NF, NF], BF, kind="ExternalInput")
    w1_d = nc.dram_tensor("w1", [1152, KC], BF, kind="ExternalInput")
    w0_d = nc.dram_tensor("w0", [1152, 4608], BF, kind="ExternalInput")
    csc1_d = nc.dram_tensor("csc1", [P, 1024], FP32, kind="ExternalInput")
    csc0_d = nc.dram_tensor("csc0", [P, 72], FP32, kind="ExternalInput")
    out_d = nc.dram_tensor("out", [36, P, KC], FP32, kind="ExternalOutput")

    # (group base chunk in mf/xb, n chunks, w1 col base)
    GROUPS = [(PP0, 9, 0), (PM0, 8, 128), (MM0, 8, 256), (MP0, 8, 384)]

    with tile.TileContext(nc) as tc:
      for _rep in range(reps):
        with (
            tc.tile_pool(name="mq", bufs=1) as mqpool,
            tc.tile_pool(name="consts", bufs=1) as cpool,
            tc.tile_pool(name="w0pool", bufs=3) as w0pool,
            tc.tile_pool(name="opool", bufs=4) as opool,
        ):
            mf = mqpool.tile([P, NCH, 512], BF)
            w1t = cpool.tile([P, 9, KC], BF)
            csc1t = cpool.tile([P, 4, 256], FP32)
            csc0t = cpool.tile([P, 72], FP32)
            nc.sync.dma_start(
                w1t[:], w1_d[:].rearrange("(c p) k -> p c k", p=P))
            nc.sync.dma_start(
                csc1t[:], csc1_d[:].rearrange("p (g k) -> p g k", g=4))
            nc.sync.dma_start(csc0t[:], csc0_d[:])

            with (
                tc.tile_pool(name="xpool", bufs=3) as xpool,
                tc.tile_pool(name="tpool", bufs=8) as tpool,
                tc.tile_pool(name="ps1", bufs=4, space="PSUM") as ps1,
            ):
                nb = 0
                for bt in range(9):          # 8 x 512 + 1 x 128 b-columns
                    bw = 512 if bt < 8 else 128
                    xb = xpool.tile([P, NCH, bw], BF)
                    nc.sync.dma_start(
                        xb[:],
                        xqq_d[:, bt * 512:bt * 512 + bw].rearrange(
                            "(c p) b -> p c b", p=P))
                    for qq in range(bw // P):
                        bsl = slice(qq * P, (qq + 1) * P)
                        psi = ps1.tile([P, KC], FP32)
                        for (gb, gn, gc) in GROUPS:
                            for a in range(gn):
                                nc.tensor.matmul(
                                    psi[:, gc:gc + 128],
                                    xb[:, gb + a, bsl],
                                    w1t[:, a, gc:gc + 128],
                                    start=(a == 0), stop=(a == gn - 1))
                        vc, vs = psi[:, 0:256], psi[:, 256:512]
                        for st in range(2):
                            t1 = tpool.tile([P, 256], FP32)
                            t2 = tpool.tile([P, 256], FP32)
                            nc.vector.tensor_tensor(
                                t1[:], vc, csc1t[:, 2 * st, :], OP.mult)
                            nc.vector.tensor_tensor(
                                t2[:], vs, csc1t[:, 2 * st + 1, :], OP.mult)
                            nc.gpsimd.tensor_tensor(
                                mf[:, nb, st * 256:(st + 1) * 256],
                                t1[:], t2[:], OP.add)
                        nb += 1

            with (
                tc.tile_pool(name="t2pool", bufs=4) as t2pool,
                tc.tile_pool(name="ps2", bufs=4, space="PSUM") as ps2,
            ):
                SBASE = {0: (PP0, 9, MM0, 8), 1: (PM0, 8, MP0, 8)}
                for par in range(2):
                    cb, cn, sb, sn = SBASE[par]
                    for t in range(9):
                        wt = w0pool.tile([P, 9, 256], BF)
                        nc.sync.dma_start(
                            wt[:],
                            w0_d[:, (par * 9 + t) * 256:
                                 (par * 9 + t + 1) * 256].rearrange(
                                     "(c p) l -> p c l", p=P))
                        pc = ps2.tile([P, KC], FP32)
                        ps = ps2.tile([P, KC], FP32)
                        for a in range(cn):
                            nc.tensor.matmul(
                                pc[:], wt[:, a, 0:128], mf[:, cb + a, :],
                                start=(a == 0), stop=(a == cn - 1))
                        for a in range(sn):
                            nc.tensor.matmul(
                                ps[:], wt[:, a, 128:256], mf[:, sb + a, :],
                                start=(a == 0), stop=(a == sn - 1))
                        for st in range(2):
                            col = ((par * 9 + t) * 2 + st) * 2
                            ts = t2pool.tile([P, KC], FP32)
                            ot = opool.tile([P, KC], FP32)
                            nc.scalar.activation(
                                ts[:], ps[:], AF.Copy,
                                scale=csc0t[:, col + 1:col + 2])
                            nc.vector.scalar_tensor_tensor(
                                ot[:], pc[:], csc0t[:, col:col + 1],
                                ts[:], OP.mult, OP.add)
                            nc.sync.dma_start(
                                out_d[(par * 9 + t) * 2 + st], ot[:])

    nc.compile()
    return nc


def _get_nc(reps=1):
    key = f"nc{reps}"
    if key not in _NC_CACHE:
        _NC_CACHE[key] = _build_nc(reps)
    return _NC_CACHE[key]


def _make_in_maps(x, expk0, expk1):
    return _prep(x, expk0, expk1)


def kernel(x, expk0, expk1):
    from concourse.bass_utils import run_bass_kernel_spmd

    in_maps = _prep(x, expk0, expk1)
    nc = _get_nc()
    try:
        res = run_bass_kernel_spmd(nc, in_maps, core_ids=list(range(NCORES)))
    except Exception:
        res = run_bass_kernel_spmd(nc, in_maps, core_ids=list(range(NCORES)))
    return _assemble(x, expk0, expk1,
                     [res.results[c]["out"] for c in range(NCORES)])


if __name__ == "__main__":
    import jax
    jax.config.update("jax_default_device", jax.devices("cpu")[0])
    import reference

    inputs = reference.setup_inputs()
    x = np.asarray(inputs["x"])
    e0 = np.asarray(inputs["expk0"])
    e1 = np.asarray(inputs["expk1"])
    expected = np.asarray(reference.reference(**inputs))
    got = _host_sim(x, e0, e1)
    err = np.max(np.abs(got - expected)) / np.max(np.abs(expected))
    print("host-sim rel err:", err)
